# revision 1
# baseline (speedup 1.0000x reference)
"""Trainium2 Bass kernel for nn_MultiHeadedAttention_44624710205499.

Reference computation (B=4, S=2048, D=512, H=8, dk=64, L=5):
  q = local_pool(query, 5)                    # causal 5-window softmax pooling
  k = local_pool(key @ W_fk + b_fk, 5)
  v = value @ W0 + b0
  x = MHA(q, k, v)   (full softmax, no mask)
  out = x @ Wout + bout

Sharding: 8 cores = (batch b = c//2) x (query-half = c%2).

v2 changes vs baseline (HW-profile driven):
  - All matmul operands bf16 (power throttle dominated the baseline: the PE
    was in a 50%-duty throttle state 58% of the time with f32r operands;
    bf16 halves SBUF read energy and PE datapath power).
  - exp outputs bf16 (halves ACT output traffic; range fits: scores <~30).
  - Inputs shipped as bf16 (halves host->device transfer).
  - SDPA normalization: per-(h,qc) DVE reciprocal then PE ones-matmul
    broadcast (kills the DMA row hop + gpsimd quirk path of the baseline).
  - Pooling software-pipelined one block deep (scores of block t+1 issue
    before denominator/PV of block t, so the PE never waits on ACT).
  - Output written bf16, host upcasts (tolerance 2e-2 vs bf16 ~4e-3).
"""

import math
import os

import ml_dtypes
import numpy as np

import concourse.bass as bass
import concourse.tile as tile
from concourse import bacc, mybir
from concourse import bass_utils

P = 128
B, S, D, H, DK, L = 4, 2048, 512, 8, 64, 5
SQ = S // 2            # query rows per core
NKI = D // P           # 4 contraction chunks of 128
SPAD = S + (L - 1)     # 2052 zero-front-padded kf length
SQPAD = SQ + (L - 1)   # 1028 query halo length
BLK = 512              # pooling block (positions per block)
NCH = 5                # ctx chunks per pooling block: 4x128 + 4
NBK = S // BLK         # 4 kf pooling blocks
NBQ = SQ // BLK        # 2 q pooling blocks
NQC = SQ // BLK        # 2 SDPA query chunks of 512
NKC = S // P           # 16 SDPA key chunks of 128
RSQD = 1.0 / math.sqrt(D)
RSQK = 1.0 / math.sqrt(DK)
NCORES = 8

F32 = mybir.dt.float32
BF16 = mybir.dt.bfloat16
F32R = mybir.dt.float32r

_PROG_CACHE = {}


def build_program(cfg=None):
    cfg = dict(cfg or {})
    MDT = BF16                       # matmul operand dtype everywhere

    nc = bacc.Bacc(
        "TRN2",
        target_bir_lowering=False,
        debug=False,
        enable_asserts=False,
        num_devices=NCORES,
    )

    keyT_d = nc.dram_tensor("keyT", [D, SPAD], MDT, kind="ExternalInput").ap()
    valT_d = nc.dram_tensor("valT", [D, S], MDT, kind="ExternalInput").ap()
    qT_d = nc.dram_tensor("qT", [D, SQPAD], MDT, kind="ExternalInput").ap()
    qrow_d = nc.dram_tensor("qrow", [SQPAD, D], MDT, kind="ExternalInput").ap()
    wfk_d = nc.dram_tensor("wfk", [D, D], MDT, kind="ExternalInput").ap()
    w0_d = nc.dram_tensor("w0", [D, D], MDT, kind="ExternalInput").ap()
    wout_d = nc.dram_tensor("wout", [D, D], MDT, kind="ExternalInput").ap()
    ones_d = nc.dram_tensor("ones_col", [P, 1], MDT, kind="ExternalInput").ap()
    vones_d = nc.dram_tensor("vones", [P, 2, 8 * H], MDT, kind="ExternalInput").ap()
    bfk_col_d = nc.dram_tensor("bfk_col", [D, 1], F32, kind="ExternalInput").ap()
    bfk_row_d = nc.dram_tensor("bfk_row", [1, D], F32, kind="ExternalInput").ap()
    b0_row_d = nc.dram_tensor("b0_row", [1, D], F32, kind="ExternalInput").ap()
    bout_col_d = nc.dram_tensor("bout_col", [D, 1], F32, kind="ExternalInput").ap()
    mask_d = nc.dram_tensor("mask_band", [NCH * P, BLK], BF16, kind="ExternalInput").ap()
    kfpad_d = nc.dram_tensor("kfpad", [D, L - 1], MDT, kind="ExternalInput").ap()
    outT_d = nc.dram_tensor("outT", [D, SQ], BF16, kind="ExternalOutput").ap()

    with tile.TileContext(nc) as tc:
        with (
            tc.tile_pool(name="A", bufs=4) as pA,      # keyT -> valT -> outT
            tc.tile_pool(name="Bp", bufs=4) as pB,     # kfT -> xt4
            tc.tile_pool(name="C", bufs=4) as pC,      # kfr -> qT/qrow -> v
            tc.tile_pool(name="W", bufs=2) as pW,      # wfk -> w0 -> wout
            tc.tile_pool(name="kTp", bufs=4) as pK,
            tc.tile_pool(name="qTp", bufs=4) as pQ,
            tc.tile_pool(name="small", bufs=1) as pS,
            tc.tile_pool(name="esc", bufs=12) as pE,   # pooling exp tiles (2 blocks deep)
            tc.tile_pool(name="esb", bufs=6) as pEb,   # SDPA exp tiles
            tc.tile_pool(name="rec", bufs=8) as pR,    # recips/broadcasts/tmp
            tc.tile_pool(name="psS", bufs=2, space="PSUM") as psS,
            tc.tile_pool(name="psV", bufs=4, space="PSUM") as psV,
        ):

            # ---------------- pooling (banded attention, sw-pipelined) -------
            def emit_pool(xT_slice, xrow, out_tiles, nblocks, filler=None):
                """xT_slice(ki) -> [P, *PAD] transposed (padded) AP, bf16.
                xrow(n) -> (tile, idx) row-layout 128-row chunk n (bf16).
                out_tiles: 4 x [P, nblocks*BLK] bf16 pooled output (transposed).
                Software-pipelined: scores(t+1) issue before den/PV(t)."""
                es_blocks = []

                def emit_scores(t):
                    es = []
                    for m in range(NCH):
                        K = P if m < NCH - 1 else L - 1
                        ps = psS.tile([P, BLK], F32, tag="psS")
                        for ki in range(NKI):
                            xa = xT_slice(ki)
                            nc.tensor.matmul(
                                ps[0:K, :],
                                xa[:, BLK * t + P * m : BLK * t + P * m + K],
                                xa[:, L - 1 + BLK * t : L - 1 + BLK * (t + 1)],
                                start=(ki == 0),
                                stop=(ki == NKI - 1),
                            )
                        e = pE.tile([P, BLK], BF16, tag="esc")
                        nc.scalar.activation(
                            e[0:K, :], ps[0:K, :],
                            mybir.ActivationFunctionType.Exp, scale=RSQD,
                        )
                        nc.vector.tensor_mul(e[0:K, :], e[0:K, :], mask_sb[0:K, m, :])
                        es.append(e)
                    return es

                def emit_den(t, es):
                    dn = psV.tile([1, BLK], F32, tag="psV", name="den")
                    for m in range(NCH):
                        K = P if m < NCH - 1 else L - 1
                        nc.tensor.matmul(
                            dn[:],
                            ones_sb[0:K, :],
                            es[m][0:K, :],
                            start=(m == 0),
                            stop=(m == NCH - 1),
                        )
                    rc = pR.tile([1, BLK], F32, tag="rec")
                    nc.vector.reciprocal(rc[:], dn[:])
                    rb = pR.tile([P, BLK], F32, tag="rec")
                    nc.gpsimd.partition_broadcast(rb[:], rc[:])
                    return rb

                def emit_pv(t, es, rb):
                    for mo in range(NKI):
                        pv = psV.tile([P, BLK], F32, tag="psV", name=f"pv{mo}")
                        for m in range(NCH):
                            K = P if m < NCH - 1 else L - 1
                            t_, idx = xrow(4 * t + m)
                            nc.tensor.matmul(
                                pv[:],
                                t_[0:K, idx, P * mo : P * (mo + 1)],
                                es[m][0:K, :],
                                start=(m == 0),
                                stop=(m == NCH - 1),
                            )
                        nc.vector.tensor_mul(
                            out_tiles[mo][:, BLK * t : BLK * (t + 1)], pv[:], rb[:]
                        )

                rbs = {}
                for t in range(nblocks):
                    if t > 0:
                        rbs[t - 1] = emit_den(t - 1, es_blocks[t - 1])
                    es_blocks.append(emit_scores(t))
                    if filler is not None:
                        filler(t)
                    if t > 0:
                        emit_pv(t - 1, es_blocks[t - 1], rbs[t - 1])
                rb_l = emit_den(nblocks - 1, es_blocks[nblocks - 1])
                emit_pv(nblocks - 1, es_blocks[nblocks - 1], rb_l)

            # ---------------- q loads first (the first pooling block waits
            # only on these), then the small constants ----------------
            qT_all = pC.tile([P, NKI, SQPAD], MDT, tag="C")
            qT_r = qT_d.rearrange("(t p) s -> p t s", p=P)
            nc.sync.dma_start(qT_all[:, :, 0:516], qT_r[:, :, 0:516])
            nc.sync.dma_start(qT_all[:, :, 516:SQPAD], qT_r[:, :, 516:SQPAD])
            qrowA = pC.tile([P, 9, BLK], MDT, tag="C")
            qrow_r = qrow_d[0:SQ, :].rearrange("(n p) d -> p n d", p=P)
            nc.sync.dma_start(qrowA[:, 0:4, :], qrow_r[:, 0:4, :])
            nc.sync.dma_start(qrowA[:, 4:8, :], qrow_r[:, 4:8, :])
            nc.sync.dma_start(qrowA[0:4, 8, :], qrow_d[SQ:SQPAD, :])

            # ---------------- constants / small loads ----------------
            mask_sb = pS.tile([P, NCH, BLK], BF16, tag="mask")
            nc.sync.dma_start(mask_sb[:], mask_d.rearrange("(m p) i -> p m i", p=P))
            bfk_col = pS.tile([P, NKI, 1], F32, tag="bfkc")
            nc.sync.dma_start(bfk_col[:], bfk_col_d.rearrange("(k p) o -> p k o", p=P))
            bout_col = pS.tile([P, NKI, 1], F32, tag="boutc")
            nc.sync.dma_start(bout_col[:], bout_col_d.rearrange("(k p) o -> p k o", p=P))
            ones_sb = pS.tile([P, 1], MDT, tag="ones")
            nc.sync.dma_start(ones_sb[:], ones_d[:])

            bfk_row = pS.tile([1, D], F32, tag="bfkrow")
            nc.sync.dma_start(bfk_row[:], bfk_row_d[:])
            bfk_bc = pS.tile([P, D], F32, tag="bfkbc")
            nc.gpsimd.partition_broadcast(bfk_bc[:], bfk_row[:])
            # variant with the 4 pad rows zeroed (for kf_row tile 0)
            bfk_bc0 = pS.tile([P, D], F32, tag="bfkbc0")
            nc.gpsimd.partition_broadcast(bfk_bc0[:], bfk_row[:])
            nc.vector.memset(bfk_bc0[0 : L - 1, :], 0.0)

            # ---------------- keyT + wfk loads (column-chunked) --------------
            keyT = [pA.tile([P, SPAD], MDT, tag="A", name=f"keyT{t}") for t in range(NKI)]
            bounds = [0, 516, 1028, 1540, SPAD]
            for cchunk in range(NKI):
                c0, c1 = bounds[cchunk], bounds[cchunk + 1]
                for t in range(NKI):
                    nc.sync.dma_start(keyT[t][:, c0:c1], keyT_d[P * t : P * (t + 1), c0:c1])
            wfk = pW.tile([P, NKI, D], MDT, tag="W")
            nc.sync.dma_start(wfk[:], wfk_d.rearrange("(k p) n -> p k n", p=P))

            qTp = [pQ.tile([P, SQ], MDT, tag="qTp", name=f"qTp{t}") for t in range(NKI)]
            emit_pool(lambda ki: qT_all[:, ki, :], lambda n: (qrowA, n), qTp, NBQ)

            # ---------------- kfT = (key @ W_fk + b_fk).T  [D, SPAD] ----------
            kfT = [pB.tile([P, SPAD], MDT, tag="B", name=f"kfT{t}") for t in range(NKI)]
            for mo in range(NKI):
                nc.sync.dma_start(kfT[mo][:, 0 : L - 1], kfpad_d[P * mo : P * (mo + 1), :])
            for ns in range(S // BLK):
                for mo in range(NKI):
                    ps = psS.tile([P, BLK], F32, tag="psS")
                    for ki in range(NKI):
                        nc.tensor.matmul(
                            ps[:],
                            wfk[:, ki, P * mo : P * (mo + 1)],
                            keyT[ki][:, L - 1 + BLK * ns : L - 1 + BLK * (ns + 1)],
                            start=(ki == 0),
                            stop=(ki == NKI - 1),
                        )
                    nc.scalar.add(
                        kfT[mo][:, L - 1 + BLK * ns : L - 1 + BLK * (ns + 1)],
                        ps[:],
                        bfk_col[:, mo, :],
                    )

            # ---------------- kf_row  [SPAD rows, D]  (17 x 128-row tiles) ----
            kfrA = pC.tile([P, 9, BLK], MDT, tag="C")
            kfrB = pC.tile([P, 8, BLK], MDT, tag="C")

            def kfr(n):
                return (kfrA, n) if n < 9 else (kfrB, n - 9)

            NROW = SPAD // P + 1  # 17
            for n in range(NROW):
                M = P if n < NROW - 1 else SPAD - P * (NROW - 1)  # 128 or 4
                ps = psS.tile([P, BLK], F32, tag="psS")
                for ki in range(NKI):
                    nc.tensor.matmul(
                        ps[0:M, :],
                        keyT[ki][:, P * n : P * n + M],
                        wfk[:, ki, :],
                        start=(ki == 0),
                        stop=(ki == NKI - 1),
                    )
                t_, idx = kfr(n)
                bias = bfk_bc0 if n == 0 else bfk_bc
                nc.vector.tensor_add(t_[0:M, idx, :], ps[0:M, :], bias[0:M, :])

            # ---------------- v = value @ W0 + b0 prep (emitted inside kfpool)
            w0 = pW.tile([P, NKI, D], MDT, tag="W")
            nc.sync.dma_start(w0[:], w0_d.rearrange("(k p) n -> p k n", p=P))
            valT = [pA.tile([P, S], MDT, tag="A", name=f"valT{t}") for t in range(NKI)]
            for t in range(NKI):
                nc.sync.dma_start(valT[t][:], valT_d[P * t : P * (t + 1), :])
            b0_row = pS.tile([1, D], F32, tag="b0row")
            nc.sync.dma_start(b0_row[:], b0_row_d[:])
            b0_bc = pS.tile([P, D], F32, tag="b0bc")
            nc.gpsimd.partition_broadcast(b0_bc[:], b0_row[:])

            vA = pC.tile([P, 8, H, DK + 1], MDT, tag="C")
            vB = pC.tile([P, 8, H, DK + 1], MDT, tag="C")
            nc.sync.dma_start(vA[:, :, :, DK], vones_d[:, 0, :].rearrange("p (n h) -> p n h", n=8))
            nc.sync.dma_start(vB[:, :, :, DK], vones_d[:, 1, :].rearrange("p (n h) -> p n h", n=8))

            def emit_v_chunk(n):
                ps = psS.tile([P, BLK], F32, tag="psS")
                for ki in range(NKI):
                    nc.tensor.matmul(
                        ps[:],
                        valT[ki][:, P * n : P * (n + 1)],
                        w0[:, ki, :],
                        start=(ki == 0),
                        stop=(ki == NKI - 1),
                    )
                vt = vA if n < 8 else vB
                nc.vector.tensor_add(
                    vt[:, n % 8, :, 0:DK],
                    ps[:].rearrange("p (h z) -> p h z", h=H),
                    b0_bc[:].rearrange("p (h z) -> p h z", h=H),
                )

            kTp = [pK.tile([P, S], MDT, tag="kTp", name=f"kTp{t}") for t in range(NKI)]

            def v_filler(t):
                for n in range(4 * t, 4 * t + 4):
                    emit_v_chunk(n)

            emit_pool(lambda ki: kfT[ki][:], kfr, kTp, NBK, filler=v_filler)

            # ---------------- SDPA + output projection ----------------
            wout = pW.tile([P, NKI, D], MDT, tag="W")
            nc.sync.dma_start(wout[:], wout_d.rearrange("(k p) n -> p k n", p=P))
            xt4 = [pB.tile([P, SQ], MDT, tag="B", name=f"xt4_{t}") for t in range(NKI)]
            outT = [pA.tile([P, SQ], BF16, tag="A", name=f"outT{t}") for t in range(NKI)]

            def emit_outproj(qc):
                for mo in range(NKI):
                    po = psS.tile([P, BLK], F32, tag="psS")
                    for ki in range(NKI):
                        nc.tensor.matmul(
                            po[:],
                            wout[:, ki, P * mo : P * (mo + 1)],
                            xt4[ki][:, BLK * qc : BLK * (qc + 1)],
                            start=(ki == 0),
                            stop=(ki == NKI - 1),
                        )
                    nc.scalar.add(
                        outT[mo][:, BLK * qc : BLK * (qc + 1)], po[:], bout_col[:, mo, :]
                    )
                    nc.sync.dma_start(
                        outT_d[P * mo : P * (mo + 1), BLK * qc : BLK * (qc + 1)],
                        outT[mo][:, BLK * qc : BLK * (qc + 1)],
                    )

            for h in range(H):
                th, off = h // 2, DK * (h % 2)
                pxs = [psV.tile([DK + 1, BLK], F32, tag="psV", name=f"px{qc}")
                       for qc in range(NQC)]
                for kc in range(NKC):
                    ps = psS.tile([P, NQC * BLK], F32, tag="psS")
                    for qc in range(NQC):
                        nc.tensor.matmul(
                            ps[:, BLK * qc : BLK * (qc + 1)],
                            kTp[th][off : off + DK, P * kc : P * (kc + 1)],
                            qTp[th][off : off + DK, BLK * qc : BLK * (qc + 1)],
                            start=True,
                            stop=True,
                        )
                    e = pEb.tile([P, NQC * BLK], BF16, tag="esb")
                    nc.scalar.activation(
                        e[:], ps[:], mybir.ActivationFunctionType.Exp, scale=RSQK
                    )
                    vt = vA if kc < 8 else vB
                    for qc in range(NQC):
                        nc.tensor.matmul(
                            pxs[qc][:],
                            vt[:, kc % 8, h, :],
                            e[:, BLK * qc : BLK * (qc + 1)],
                            start=(kc == 0),
                            stop=(kc == NKC - 1),
                        )
                def norm_qc(qc):
                    px = pxs[qc]
                    # partition_broadcast HW ucode reads tile partition 0,
                    # not the AP base -- reciprocal at base 64 (lane-aligned),
                    # then DMA the row down to a base-0 tile for the broadcast.
                    rc = pR.tile([DK + 1, BLK], F32, tag="rec")
                    nc.vector.reciprocal(rc[DK : DK + 1, :], px[DK : DK + 1, :])
                    rc0 = pR.tile([1, BLK], F32, tag="rec")
                    nc.sync.dma_start(rc0[:], rc[DK : DK + 1, :])
                    rb = pR.tile([DK, BLK], F32, tag="rec")
                    nc.gpsimd.partition_broadcast(rb[:], rc0[:])
                    if h % 2 == 0:
                        nc.vector.tensor_mul(
                            xt4[th][0:DK, BLK * qc : BLK * (qc + 1)], px[0:DK, :], rb[:]
                        )
                    else:
                        tmp = pR.tile([DK, BLK], MDT, tag="rec")
                        nc.vector.tensor_mul(tmp[:], px[0:DK, :], rb[:])
                        nc.sync.dma_start(
                            xt4[th][DK:P, BLK * qc : BLK * (qc + 1)], tmp[:]
                        )

                norm_qc(0)
                if h == H - 1:
                    emit_outproj(0)
                norm_qc(1)
            emit_outproj(1)

    nc.compile()
    return nc


def make_band_mask():
    j = np.arange(NCH * P)[:, None]
    i = np.arange(BLK)[None, :]
    return (((j - i) >= 0) & ((j - i) <= L - 1)).astype(np.float32)


def make_core_inputs(query, key, value, W_fk, b_fk, W0, b0, Wout, bout, cfg=None):
    """Build the 8 per-core input dicts from full inputs (host-side shard)."""
    bf = ml_dtypes.bfloat16
    shared = {
        "wfk": np.ascontiguousarray(W_fk).astype(bf),
        "w0": np.ascontiguousarray(W0).astype(bf),
        "wout": np.ascontiguousarray(Wout).astype(bf),
        "ones_col": np.ones((P, 1), bf),
        "vones": np.ones((P, 2, 8 * H), bf),
        "bfk_col": np.ascontiguousarray(b_fk.reshape(D, 1), np.float32),
        "bfk_row": np.ascontiguousarray(b_fk.reshape(1, D), np.float32),
        "b0_row": np.ascontiguousarray(b0.reshape(1, D), np.float32),
        "bout_col": np.ascontiguousarray(bout.reshape(D, 1), np.float32),
        "mask_band": make_band_mask().astype(bf),
        "kfpad": np.zeros((D, L - 1), bf),
    }
    in_maps = []
    for c in range(NCORES):
        b, half = divmod(c, 2)
        q0 = half * SQ
        q_halo = np.zeros((SQPAD, D), np.float32)
        lo = max(0, q0 - (L - 1))
        q_halo[(L - 1) - (q0 - lo):] = query[b, lo : q0 + SQ]
        keyT_pad = np.zeros((D, SPAD), np.float32)
        keyT_pad[:, L - 1 :] = key[b].T
        m = dict(shared)
        m["keyT"] = keyT_pad.astype(bf)
        m["valT"] = np.ascontiguousarray(value[b].T).astype(bf)
        m["qT"] = np.ascontiguousarray(q_halo.T).astype(bf)
        m["qrow"] = q_halo.astype(bf)
        in_maps.append(m)
    return in_maps


def _cfg_from_env():
    return {}


def get_program(cfg=None):
    cfg = dict(cfg or {})
    key_t = tuple(sorted(cfg.items()))
    if key_t not in _PROG_CACHE:
        _PROG_CACHE[key_t] = build_program(cfg)
    return _PROG_CACHE[key_t]


def kernel(query, key, value, mask=None, W_fk=None, b_fk=None, W0=None, b0=None,
           Wout=None, bout=None, **extra):
    del mask, extra  # mask is dead in the reference (forward passes mask=None)
    cfg = _cfg_from_env()
    nc = get_program(cfg)

    query = np.asarray(query, np.float32)
    key = np.asarray(key, np.float32)
    value = np.asarray(value, np.float32)
    in_maps = make_core_inputs(
        query, key, value,
        np.asarray(W_fk, np.float32), np.asarray(b_fk, np.float32),
        np.asarray(W0, np.float32), np.asarray(b0, np.float32),
        np.asarray(Wout, np.float32), np.asarray(bout, np.float32),
        cfg,
    )
    res = bass_utils.run_bass_kernel_spmd(nc, in_maps, core_ids=list(range(NCORES)))
    out = np.empty((B, S, D), np.float32)
    for c in range(NCORES):
        b, half = divmod(c, 2)
        out[b, half * SQ : (half + 1) * SQ, :] = res.results[c]["outT"].astype(np.float32).T
    return out



# revision 8
# speedup vs baseline: 1.3582x; 1.3582x over previous
"""Trainium2 Bass kernel for nn_MultiHeadedAttention_44624710205499.

Reference computation (B=4, S=2048, D=512, H=8, dk=64, L=5):
  q = local_pool(query, 5)                    # causal 5-window softmax pooling
  k = local_pool(key @ W_fk + b_fk, 5)
  v = value @ W0 + b0
  x = MHA(q, k, v)   (full softmax, no mask)
  out = x @ Wout + bout

Sharding: 8 cores = (batch b = c//2) x (query-half = c%2).

v3 changes vs v2 (profile-driven; v2 = 310us/core, PE 86% busy at 46% MFU,
ACT 183us, DVE 137us of which 73us was single-lane reciprocals):
  - Banded pooling: scores/den/PV computed only on the ~132-wide diagonal
    windows of each 512 block (5x less PE/ACT/DVE work on pooling).
    PSUM regions with split start/stop flags handle window overlap.
  - SDPA scores row-packed: head pairs occupy PE row-groups 0-1 / 2-3 via
    base-partition 0/64 operands, so the two 64-contraction matmuls run
    concurrently (scores PE time halves).
  - exp batched at N=1024 per (pair, kc, qc), double-buffered PSUM so ACT
    streams back-to-back.
  - All [1,512] reciprocals -> reciprocal_approx_fast (~5x).
  - outproj bias-add moved ACT -> DVE (ACT is the SDPA-phase bottleneck).
"""

import math
import os

import ml_dtypes
import numpy as np

import concourse.bass as bass
import concourse.tile as tile
from concourse import bacc, mybir
from concourse import bass_utils

P = 128
B, S, D, H, DK, L = 4, 2048, 512, 8, 64, 5
SQ = S // 2            # query rows per core
NKI = D // P           # 4 contraction chunks of 128
SPAD = S + (L - 1)     # 2052 zero-front-padded kf length
SQPAD = SQ + (L - 1)   # 1028 query halo length
BLK = 512              # pooling block (positions per block)
NCH = 5                # ctx chunks per pooling block: 4x128 + 4
NBK = S // BLK         # 4 kf pooling blocks
NBQ = SQ // BLK        # 2 q pooling blocks
NQC = SQ // BLK        # 2 SDPA query chunks of 512
NKC = S // P           # 16 SDPA key chunks of 128
RSQD = 1.0 / math.sqrt(D)
RSQK = 1.0 / math.sqrt(DK)
NCORES = 8

F32 = mybir.dt.float32
BF16 = mybir.dt.bfloat16

# Banded pooling geometry.  For 512-block t, context chunk m covers window
# rows (padded cols) [512t+128m, +K) = positions [512t+128m-4, +K), serving
# block-relative query cols [w0, w1).  Scores/exp/mask tiles pack the five
# windows as S1 = [m0|m1|m2] (392 cols) and S2 = [m3|m4] (136 cols).
#            m   s_off  w0   w1   K    tile
SEGS = [
    (0,   0,    0,  128, 128, 0),
    (1, 128,  124,  256, 128, 0),
    (2, 260,  252,  384, 128, 0),
    (3,   0,  380,  512, 128, 1),
    (4, 132,  508,  512,   4, 1),
]
S1W, S2W = 392, 136
# PSUM accumulation pieces per m: (lo, hi, start, stop) in block-rel cols.
# Adjacent windows overlap by 4 cols; boundary cols get two accumulating
# writes (start on first, stop on last).
PIECES = [
    [(0, 124, True, True), (124, 128, True, False)],
    [(124, 128, False, True), (128, 252, True, True), (252, 256, True, False)],
    [(252, 256, False, True), (256, 380, True, True), (380, 384, True, False)],
    [(380, 384, False, True), (384, 508, True, True), (508, 512, True, False)],
    [(508, 512, False, True)],
]

_PROG_CACHE = {}


def build_program(cfg=None):
    cfg = dict(cfg or {})
    MDT = BF16                       # matmul operand dtype everywhere

    nc = bacc.Bacc(
        "TRN2",
        target_bir_lowering=False,
        debug=False,
        enable_asserts=False,
        num_devices=NCORES,
    )

    keyT_d = nc.dram_tensor("keyT", [D, SPAD], MDT, kind="ExternalInput").ap()
    valT_d = nc.dram_tensor("valT", [D, S], MDT, kind="ExternalInput").ap()
    qT_d = nc.dram_tensor("qT", [D, SQPAD], MDT, kind="ExternalInput").ap()
    qrow_d = nc.dram_tensor("qrow", [SQPAD, D], MDT, kind="ExternalInput").ap()
    wfk_d = nc.dram_tensor("wfk", [D, D], MDT, kind="ExternalInput").ap()
    w0_d = nc.dram_tensor("w0", [D, D], MDT, kind="ExternalInput").ap()
    wout_d = nc.dram_tensor("wout", [D, D], MDT, kind="ExternalInput").ap()
    ones_d = nc.dram_tensor("ones_col", [P, 1], MDT, kind="ExternalInput").ap()
    vones_d = nc.dram_tensor("vones", [P, 2, 8 * H], MDT, kind="ExternalInput").ap()
    bfk_col_d = nc.dram_tensor("bfk_col", [D, 1], F32, kind="ExternalInput").ap()
    bfk_row_d = nc.dram_tensor("bfk_row", [1, D], F32, kind="ExternalInput").ap()
    b0_row_d = nc.dram_tensor("b0_row", [1, D], F32, kind="ExternalInput").ap()
    bout_col_d = nc.dram_tensor("bout_col", [D, 1], F32, kind="ExternalInput").ap()
    mask_d = nc.dram_tensor("mask_band", [P, S1W + S2W], BF16, kind="ExternalInput").ap()
    kfpad_d = nc.dram_tensor("kfpad", [D, L - 1], MDT, kind="ExternalInput").ap()
    outT_d = nc.dram_tensor("outT", [D, SQ], BF16, kind="ExternalOutput").ap()

    with tile.TileContext(nc) as tc:
        with (
            tc.tile_pool(name="A", bufs=4) as pA,      # keyT -> valT -> outT
            tc.tile_pool(name="Bp", bufs=4) as pB,     # kfT -> xt4
            tc.tile_pool(name="C", bufs=4) as pC,      # kfr -> qT/qrow -> v
            tc.tile_pool(name="W", bufs=2) as pW,      # wfk -> w0 -> wout
            tc.tile_pool(name="kTp", bufs=4) as pK,
            tc.tile_pool(name="qTp", bufs=4) as pQ,
            tc.tile_pool(name="small", bufs=1) as pS,
            tc.tile_pool(name="esc", bufs=12) as pE,   # pooling exp tiles
            tc.tile_pool(name="esb", bufs=6) as pEb,   # SDPA exp tiles
            tc.tile_pool(name="rec", bufs=8) as pR,    # recips/broadcasts/tmp
            tc.tile_pool(name="psS", bufs=2, space="PSUM") as psS,
            tc.tile_pool(name="psV", bufs=4, space="PSUM") as psV,
        ):

            # ---------------- pooling (banded attention, sw-pipelined) -------
            def emit_pool(xT_slice, xrow, out_tiles, nblocks, filler=None):
                """xT_slice(ki) -> [P, *PAD] transposed (padded) AP, bf16.
                xrow(n) -> (tile, idx) row-layout 128-row chunk n (bf16).
                out_tiles: 4 x [P, nblocks*BLK] bf16 pooled output (transposed).
                Banded: each m computes only its [K, w1-w0] window.
                Software-pipelined: scores(t+1) issue before den/PV(t)."""
                es_blocks = []

                def emit_scores(t):
                    Sp = [psS.tile([P, S1W], F32, tag="psS", name="S1"),
                          psS.tile([P, S2W], F32, tag="psS", name="S2")]
                    for (m, so, w0, w1, K, sel) in SEGS:
                        Ss = Sp[sel]
                        for ki in range(NKI):
                            xa = xT_slice(ki)
                            nc.tensor.matmul(
                                Ss[0:K, so : so + (w1 - w0)],
                                xa[:, BLK * t + P * m : BLK * t + P * m + K],
                                xa[:, L - 1 + BLK * t + w0 : L - 1 + BLK * t + w1],
                                start=(ki == 0),
                                stop=(ki == NKI - 1),
                            )
                    e1 = pE.tile([P, S1W], BF16, tag="esc", name="e1")
                    e2 = pE.tile([P, S2W], BF16, tag="esc", name="e2")
                    nc.scalar.activation(
                        e1[:], Sp[0][:], mybir.ActivationFunctionType.Exp, scale=RSQD)
                    nc.scalar.activation(
                        e2[:, 0:132], Sp[1][:, 0:132],
                        mybir.ActivationFunctionType.Exp, scale=RSQD)
                    nc.scalar.activation(
                        e2[0:4, 132:136], Sp[1][0:4, 132:136],
                        mybir.ActivationFunctionType.Exp, scale=RSQD)
                    nc.vector.tensor_mul(e1[:], e1[:], mask_sb[:, 0:S1W])
                    nc.vector.tensor_mul(
                        e2[:, 0:132], e2[:, 0:132], mask_sb[:, S1W : S1W + 132])
                    nc.vector.tensor_mul(
                        e2[0:4, 132:136], e2[0:4, 132:136],
                        mask_sb[0:4, S1W + 132 : S1W + 136])
                    return (e1, e2)

                def es_slice(es, m, lo, hi):
                    _, so, w0, _, K, sel = SEGS[m]
                    c0 = so + (lo - w0)
                    return es[sel][0:K, c0 : c0 + (hi - lo)]

                def emit_den(t, es):
                    dn = psV.tile([1, BLK], F32, tag="psV", name="den")
                    for m in range(NCH):
                        K = SEGS[m][4]
                        for (lo, hi, st, sp) in PIECES[m]:
                            nc.tensor.matmul(
                                dn[:, lo:hi],
                                ones_sb[0:K, :],
                                es_slice(es, m, lo, hi),
                                start=st,
                                stop=sp,
                            )
                    rc = pR.tile([1, BLK], F32, tag="rec")
                    nc.vector.reciprocal_approx_fast(rc[:], dn[:])
                    rb = pR.tile([P, BLK], F32, tag="rec")
                    nc.gpsimd.partition_broadcast(rb[:], rc[:])
                    return rb

                def emit_pv(t, es, rb):
                    for mo in range(NKI):
                        pv = psV.tile([P, BLK], F32, tag="psV", name=f"pv{mo}")
                        for m in range(NCH):
                            K = SEGS[m][4]
                            t_, idx = xrow(4 * t + m)
                            for (lo, hi, st, sp) in PIECES[m]:
                                nc.tensor.matmul(
                                    pv[:, lo:hi],
                                    t_[0:K, idx, P * mo : P * (mo + 1)],
                                    es_slice(es, m, lo, hi),
                                    start=st,
                                    stop=sp,
                                )
                        nc.vector.tensor_mul(
                            out_tiles[mo][:, BLK * t : BLK * (t + 1)], pv[:], rb[:]
                        )

                rbs = {}
                for t in range(nblocks):
                    if t > 0:
                        rbs[t - 1] = emit_den(t - 1, es_blocks[t - 1])
                    es_blocks.append(emit_scores(t))
                    if filler is not None:
                        filler(t)
                    if t > 0:
                        emit_pv(t - 1, es_blocks[t - 1], rbs[t - 1])
                rb_l = emit_den(nblocks - 1, es_blocks[nblocks - 1])
                emit_pv(nblocks - 1, es_blocks[nblocks - 1], rb_l)

            # ---------------- q loads first (the first pooling block waits
            # only on these), then the small constants ----------------
            qT_all = pC.tile([P, NKI, SQPAD], MDT, tag="C")
            qT_r = qT_d.rearrange("(t p) s -> p t s", p=P)
            nc.sync.dma_start(qT_all[:, :, 0:516], qT_r[:, :, 0:516])
            nc.sync.dma_start(qT_all[:, :, 516:SQPAD], qT_r[:, :, 516:SQPAD])
            qrowA = pC.tile([P, 9, BLK], MDT, tag="C")
            qrow_r = qrow_d[0:SQ, :].rearrange("(n p) d -> p n d", p=P)
            nc.sync.dma_start(qrowA[:, 0:4, :], qrow_r[:, 0:4, :])
            nc.sync.dma_start(qrowA[:, 4:8, :], qrow_r[:, 4:8, :])
            nc.sync.dma_start(qrowA[0:4, 8, :], qrow_d[SQ:SQPAD, :])

            # ---------------- constants / small loads ----------------
            mask_sb = pS.tile([P, S1W + S2W], BF16, tag="mask")
            nc.sync.dma_start(mask_sb[:], mask_d[:])
            bfk_col = pS.tile([P, NKI, 1], F32, tag="bfkc")
            nc.sync.dma_start(bfk_col[:], bfk_col_d.rearrange("(k p) o -> p k o", p=P))
            bout_col = pS.tile([P, NKI, 1], F32, tag="boutc")
            nc.sync.dma_start(bout_col[:], bout_col_d.rearrange("(k p) o -> p k o", p=P))
            ones_sb = pS.tile([P, 1], MDT, tag="ones")
            nc.sync.dma_start(ones_sb[:], ones_d[:])

            bfk_row = pS.tile([1, D], F32, tag="bfkrow")
            nc.sync.dma_start(bfk_row[:], bfk_row_d[:])
            bfk_bc = pS.tile([P, D], F32, tag="bfkbc")
            nc.gpsimd.partition_broadcast(bfk_bc[:], bfk_row[:])
            # variant with the 4 pad rows zeroed (for kf_row tile 0)
            bfk_bc0 = pS.tile([P, D], F32, tag="bfkbc0")
            nc.gpsimd.partition_broadcast(bfk_bc0[:], bfk_row[:])
            nc.vector.memset(bfk_bc0[0 : L - 1, :], 0.0)

            # ---------------- keyT + wfk loads (column-chunked) --------------
            keyT = [pA.tile([P, SPAD], MDT, tag="A", name=f"keyT{t}") for t in range(NKI)]
            bounds = [0, 516, 1028, 1540, SPAD]
            for cchunk in range(NKI):
                c0, c1 = bounds[cchunk], bounds[cchunk + 1]
                for t in range(NKI):
                    nc.sync.dma_start(keyT[t][:, c0:c1], keyT_d[P * t : P * (t + 1), c0:c1])
            wfk = pW.tile([P, NKI, D], MDT, tag="W")
            nc.sync.dma_start(wfk[:], wfk_d.rearrange("(k p) n -> p k n", p=P))

            qTp = [pQ.tile([P, SQ], MDT, tag="qTp", name=f"qTp{t}") for t in range(NKI)]
            emit_pool(lambda ki: qT_all[:, ki, :], lambda n: (qrowA, n), qTp, NBQ)

            # ---------------- kfT = (key @ W_fk + b_fk).T  [D, SPAD] ----------
            kfT = [pB.tile([P, SPAD], MDT, tag="B", name=f"kfT{t}") for t in range(NKI)]
            for mo in range(NKI):
                nc.sync.dma_start(kfT[mo][:, 0 : L - 1], kfpad_d[P * mo : P * (mo + 1), :])
            for ns in range(S // BLK):
                for mo in range(NKI):
                    ps = psS.tile([P, BLK], F32, tag="psS")
                    for ki in range(NKI):
                        nc.tensor.matmul(
                            ps[:],
                            wfk[:, ki, P * mo : P * (mo + 1)],
                            keyT[ki][:, L - 1 + BLK * ns : L - 1 + BLK * (ns + 1)],
                            start=(ki == 0),
                            stop=(ki == NKI - 1),
                        )
                    nc.scalar.add(
                        kfT[mo][:, L - 1 + BLK * ns : L - 1 + BLK * (ns + 1)],
                        ps[:],
                        bfk_col[:, mo, :],
                    )

            # ---------------- kf_row  [SPAD rows, D]  (17 x 128-row tiles) ----
            kfrA = pC.tile([P, 9, BLK], MDT, tag="C")
            kfrB = pC.tile([P, 8, BLK], MDT, tag="C")

            def kfr(n):
                return (kfrA, n) if n < 9 else (kfrB, n - 9)

            NROW = SPAD // P + 1  # 17
            for n in range(NROW):
                M = P if n < NROW - 1 else SPAD - P * (NROW - 1)  # 128 or 4
                ps = psS.tile([P, BLK], F32, tag="psS")
                for ki in range(NKI):
                    nc.tensor.matmul(
                        ps[0:M, :],
                        keyT[ki][:, P * n : P * n + M],
                        wfk[:, ki, :],
                        start=(ki == 0),
                        stop=(ki == NKI - 1),
                    )
                t_, idx = kfr(n)
                bias = bfk_bc0 if n == 0 else bfk_bc
                nc.vector.tensor_add(t_[0:M, idx, :], ps[0:M, :], bias[0:M, :])

            # ---------------- v = value @ W0 + b0 prep (emitted inside kfpool)
            w0 = pW.tile([P, NKI, D], MDT, tag="W")
            nc.sync.dma_start(w0[:], w0_d.rearrange("(k p) n -> p k n", p=P))
            valT = [pA.tile([P, S], MDT, tag="A", name=f"valT{t}") for t in range(NKI)]
            for t in range(NKI):
                nc.sync.dma_start(valT[t][:], valT_d[P * t : P * (t + 1), :])
            b0_row = pS.tile([1, D], F32, tag="b0row")
            nc.sync.dma_start(b0_row[:], b0_row_d[:])
            b0_bc = pS.tile([P, D], F32, tag="b0bc")
            nc.gpsimd.partition_broadcast(b0_bc[:], b0_row[:])

            vA = pC.tile([P, 8, H, DK + 1], MDT, tag="C")
            vB = pC.tile([P, 8, H, DK + 1], MDT, tag="C")
            nc.sync.dma_start(vA[:, :, :, DK], vones_d[:, 0, :].rearrange("p (n h) -> p n h", n=8))
            nc.sync.dma_start(vB[:, :, :, DK], vones_d[:, 1, :].rearrange("p (n h) -> p n h", n=8))

            def emit_v_chunk(n):
                ps = psS.tile([P, BLK], F32, tag="psS")
                for ki in range(NKI):
                    nc.tensor.matmul(
                        ps[:],
                        valT[ki][:, P * n : P * (n + 1)],
                        w0[:, ki, :],
                        start=(ki == 0),
                        stop=(ki == NKI - 1),
                    )
                vt = vA if n < 8 else vB
                nc.vector.tensor_add(
                    vt[:, n % 8, :, 0:DK],
                    ps[:].rearrange("p (h z) -> p h z", h=H),
                    b0_bc[:].rearrange("p (h z) -> p h z", h=H),
                )

            kTp = [pK.tile([P, S], MDT, tag="kTp", name=f"kTp{t}") for t in range(NKI)]

            def v_filler(t):
                for n in range(4 * t, 4 * t + 4):
                    emit_v_chunk(n)

            emit_pool(lambda ki: kfT[ki][:], kfr, kTp, NBK, filler=v_filler)

            # ---------------- SDPA + output projection ----------------
            wout = pW.tile([P, NKI, D], MDT, tag="W")
            nc.sync.dma_start(wout[:], wout_d.rearrange("(k p) n -> p k n", p=P))
            xt4 = [pB.tile([P, SQ], MDT, tag="B", name=f"xt4_{t}") for t in range(NKI)]
            outT = [pA.tile([P, SQ], BF16, tag="A", name=f"outT{t}") for t in range(NKI)]

            def emit_outproj(qc):
                for mo in range(NKI):
                    po = psS.tile([P, BLK], F32, tag="psS")
                    for ki in range(NKI):
                        nc.tensor.matmul(
                            po[:],
                            wout[:, ki, P * mo : P * (mo + 1)],
                            xt4[ki][:, BLK * qc : BLK * (qc + 1)],
                            start=(ki == 0),
                            stop=(ki == NKI - 1),
                        )
                    nc.vector.tensor_scalar_add(
                        outT[mo][:, BLK * qc : BLK * (qc + 1)], po[:], bout_col[:, mo, :]
                    )
                    nc.sync.dma_start(
                        outT_d[P * mo : P * (mo + 1), BLK * qc : BLK * (qc + 1)],
                        outT[mo][:, BLK * qc : BLK * (qc + 1)],
                    )

            if cfg.get("v2_sdpa"):
                # v2 SDPA path (sequential heads, DVE reciprocal) for bisection
                for h in range(H):
                    th, off = h // 2, DK * (h % 2)
                    pxs = [psV.tile([DK + 1, BLK], F32, tag="psV", name=f"px{qc}")
                           for qc in range(NQC)]
                    for kc in range(NKC):
                        ps = psS.tile([P, NQC * BLK], F32, tag="psS")
                        for qc in range(NQC):
                            nc.tensor.matmul(
                                ps[:, BLK * qc : BLK * (qc + 1)],
                                kTp[th][off : off + DK, P * kc : P * (kc + 1)],
                                qTp[th][off : off + DK, BLK * qc : BLK * (qc + 1)],
                                start=True,
                                stop=True,
                            )
                        e = pEb.tile([P, NQC * BLK], BF16, tag="esb")
                        nc.scalar.activation(
                            e[:], ps[:], mybir.ActivationFunctionType.Exp, scale=RSQK
                        )
                        vt = vA if kc < 8 else vB
                        for qc in range(NQC):
                            nc.tensor.matmul(
                                pxs[qc][:],
                                vt[:, kc % 8, h, :],
                                e[:, BLK * qc : BLK * (qc + 1)],
                                start=(kc == 0),
                                stop=(kc == NKC - 1),
                            )
                    def norm_qc2(qc):
                        px = pxs[qc]
                        rc = pR.tile([DK + 1, BLK], F32, tag="rec")
                        nc.vector.reciprocal(rc[DK : DK + 1, :], px[DK : DK + 1, :])
                        rc0 = pR.tile([1, BLK], F32, tag="rec")
                        nc.sync.dma_start(rc0[:], rc[DK : DK + 1, :])
                        rb = pR.tile([DK, BLK], F32, tag="rec")
                        nc.gpsimd.partition_broadcast(rb[:], rc0[:])
                        if h % 2 == 0:
                            nc.vector.tensor_mul(
                                xt4[th][0:DK, BLK * qc : BLK * (qc + 1)], px[0:DK, :], rb[:]
                            )
                        else:
                            tmp = pR.tile([DK, BLK], MDT, tag="rec")
                            nc.vector.tensor_mul(tmp[:], px[0:DK, :], rb[:])
                            nc.sync.dma_start(
                                xt4[th][DK:P, BLK * qc : BLK * (qc + 1)], tmp[:]
                            )
                    norm_qc2(0)
                    if h == H - 1:
                        emit_outproj(0)
                    norm_qc2(1)
                emit_outproj(1)

            # Head pairs (2p, 2p+1): lhsT/rhs slices at base partitions 0/64
            # land in disjoint PE row-groups -> the two 64-contraction score
            # matmuls run concurrently (tile_position auto-derived).
            for p in range(4 if not cfg.get("v2_sdpa") else 0):
                pxs = {}
                for hh in range(2):
                    for qc in range(NQC):
                        pxs[(hh, qc)] = psV.tile(
                            [DK + 1, BLK], F32, tag="psV", name=f"px{hh}{qc}")
                for kc in range(NKC):
                    vt = vA if kc < 8 else vB
                    for qc in range(NQC):
                        ps = psS.tile([P, 2 * BLK], F32, tag="psS")
                        nc.tensor.matmul(
                            ps[:, 0:BLK],
                            kTp[p][0:DK, P * kc : P * (kc + 1)],
                            qTp[p][0:DK, BLK * qc : BLK * (qc + 1)],
                            start=True,
                            stop=True,
                        )
                        nc.tensor.matmul(
                            ps[:, BLK : 2 * BLK],
                            kTp[p][DK:P, P * kc : P * (kc + 1)],
                            qTp[p][DK:P, BLK * qc : BLK * (qc + 1)],
                            start=True,
                            stop=True,
                        )
                        e = pEb.tile([P, 2 * BLK], BF16, tag="esb")
                        nc.scalar.activation(
                            e[:], ps[:], mybir.ActivationFunctionType.Exp, scale=RSQK
                        )
                        nc.tensor.matmul(
                            pxs[(0, qc)][:],
                            vt[:, kc % 8, 2 * p, :],
                            e[:, 0:BLK],
                            start=(kc == 0),
                            stop=(kc == NKC - 1),
                        )
                        nc.tensor.matmul(
                            pxs[(1, qc)][:],
                            vt[:, kc % 8, 2 * p + 1, :],
                            e[:, BLK : 2 * BLK],
                            start=(kc == 0),
                            stop=(kc == NKC - 1),
                        )

                def norm_qc(hh, qc):
                    px = pxs[(hh, qc)]
                    # custom-ucode DVE/gpsimd ops address tile partition 0,
                    # not the AP base -- native-copy the den row out of PSUM
                    # (same partition), DMA it down to a base-0 tile, then
                    # reciprocal + broadcast at base 0.
                    dnc = pR.tile([DK + 1, BLK], F32, tag="rec")
                    nc.vector.tensor_copy(dnc[DK : DK + 1, :], px[DK : DK + 1, :])
                    dn0 = pR.tile([1, BLK], F32, tag="rec")
                    nc.sync.dma_start(dn0[:], dnc[DK : DK + 1, :])
                    rc0 = pR.tile([1, BLK], F32, tag="rec")
                    nc.vector.reciprocal_approx_fast(rc0[:], dn0[:])
                    rb = pR.tile([DK, BLK], F32, tag="rec")
                    nc.gpsimd.partition_broadcast(rb[:], rc0[:])
                    if hh == 0:
                        nc.vector.tensor_mul(
                            xt4[p][0:DK, BLK * qc : BLK * (qc + 1)], px[0:DK, :], rb[:]
                        )
                    else:
                        tmp = pR.tile([DK, BLK], MDT, tag="rec")
                        nc.vector.tensor_mul(tmp[:], px[0:DK, :], rb[:])
                        nc.sync.dma_start(
                            xt4[p][DK:P, BLK * qc : BLK * (qc + 1)], tmp[:]
                        )

                norm_qc(0, 0)
                norm_qc(1, 0)
                if p == 3:
                    emit_outproj(0)
                norm_qc(0, 1)
                norm_qc(1, 1)
            if not cfg.get("v2_sdpa"):
                emit_outproj(1)

    nc.compile()
    return nc


def make_band_mask():
    """[128, 528] bf16: per-segment band masks, S1 | S2 packed layout."""
    M = np.zeros((P, S1W + S2W), np.float32)
    off = {0: 0, 1: S1W}
    for (m, so, w0, w1, K, sel) in SEGS:
        for r in range(K):
            j = 128 * m - 4 + r
            for c in range(w1 - w0):
                i = w0 + c
                if 0 <= i - j <= L - 1:
                    M[r, off[sel] + so + c] = 1.0
    return M


def make_core_inputs(query, key, value, W_fk, b_fk, W0, b0, Wout, bout, cfg=None):
    """Build the 8 per-core input dicts from full inputs (host-side shard)."""
    bf = ml_dtypes.bfloat16
    shared = {
        "wfk": np.ascontiguousarray(W_fk).astype(bf),
        "w0": np.ascontiguousarray(W0).astype(bf),
        "wout": np.ascontiguousarray(Wout).astype(bf),
        "ones_col": np.ones((P, 1), bf),
        "vones": np.ones((P, 2, 8 * H), bf),
        "bfk_col": np.ascontiguousarray(b_fk.reshape(D, 1), np.float32),
        "bfk_row": np.ascontiguousarray(b_fk.reshape(1, D), np.float32),
        "b0_row": np.ascontiguousarray(b0.reshape(1, D), np.float32),
        "bout_col": np.ascontiguousarray(bout.reshape(D, 1), np.float32),
        "mask_band": make_band_mask().astype(bf),
        "kfpad": np.zeros((D, L - 1), bf),
    }
    in_maps = []
    for c in range(NCORES):
        b, half = divmod(c, 2)
        q0 = half * SQ
        q_halo = np.zeros((SQPAD, D), np.float32)
        lo = max(0, q0 - (L - 1))
        q_halo[(L - 1) - (q0 - lo):] = query[b, lo : q0 + SQ]
        keyT_pad = np.zeros((D, SPAD), np.float32)
        keyT_pad[:, L - 1 :] = key[b].T
        m = dict(shared)
        m["keyT"] = keyT_pad.astype(bf)
        m["valT"] = np.ascontiguousarray(value[b].T).astype(bf)
        m["qT"] = np.ascontiguousarray(q_halo.T).astype(bf)
        m["qrow"] = q_halo.astype(bf)
        in_maps.append(m)
    return in_maps


def _cfg_from_env():
    cfg = {}
    if os.environ.get("K_V2_SDPA"):
        cfg["v2_sdpa"] = True
    return cfg


def get_program(cfg=None):
    cfg = dict(cfg or {})
    key_t = tuple(sorted(cfg.items()))
    if key_t not in _PROG_CACHE:
        _PROG_CACHE[key_t] = build_program(cfg)
    return _PROG_CACHE[key_t]


def kernel(query, key, value, mask=None, W_fk=None, b_fk=None, W0=None, b0=None,
           Wout=None, bout=None, **extra):
    del mask, extra  # mask is dead in the reference (forward passes mask=None)
    cfg = _cfg_from_env()
    nc = get_program(cfg)

    query = np.asarray(query, np.float32)
    key = np.asarray(key, np.float32)
    value = np.asarray(value, np.float32)
    in_maps = make_core_inputs(
        query, key, value,
        np.asarray(W_fk, np.float32), np.asarray(b_fk, np.float32),
        np.asarray(W0, np.float32), np.asarray(b0, np.float32),
        np.asarray(Wout, np.float32), np.asarray(bout, np.float32),
        cfg,
    )
    res = bass_utils.run_bass_kernel_spmd(nc, in_maps, core_ids=list(range(NCORES)))
    out = np.empty((B, S, D), np.float32)
    for c in range(NCORES):
        b, half = divmod(c, 2)
        out[b, half * SQ : (half + 1) * SQ, :] = res.results[c]["outT"].astype(np.float32).T
    return out


# revision 10
# speedup vs baseline: 1.4795x; 1.0893x over previous
"""Trainium2 Bass kernel for nn_MultiHeadedAttention_44624710205499.

Reference computation (B=4, S=2048, D=512, H=8, dk=64, L=5):
  q = local_pool(query, 5)                    # causal 5-window softmax pooling
  k = local_pool(key @ W_fk + b_fk, 5)
  v = value @ W0 + b0
  x = MHA(q, k, v)   (full softmax, no mask)
  out = x @ Wout + bout

Sharding: 8 cores = (batch b = c//2) x (query-half = c%2).

v3 changes vs v2 (profile-driven; v2 = 310us/core, PE 86% busy at 46% MFU,
ACT 183us, DVE 137us of which 73us was single-lane reciprocals):
  - Banded pooling: scores/den/PV computed only on the ~132-wide diagonal
    windows of each 512 block (5x less PE/ACT/DVE work on pooling).
    PSUM regions with split start/stop flags handle window overlap.
  - SDPA scores row-packed: head pairs occupy PE row-groups 0-1 / 2-3 via
    base-partition 0/64 operands, so the two 64-contraction matmuls run
    concurrently (scores PE time halves).
  - exp batched at N=1024 per (pair, kc, qc), double-buffered PSUM so ACT
    streams back-to-back.
  - All [1,512] reciprocals -> reciprocal_approx_fast (~5x).
  - outproj bias-add moved ACT -> DVE (ACT is the SDPA-phase bottleneck).
"""

import math
import os

import ml_dtypes
import numpy as np

import concourse.bass as bass
import concourse.tile as tile
from concourse import bacc, mybir
from concourse import bass_utils

P = 128
B, S, D, H, DK, L = 4, 2048, 512, 8, 64, 5
SQ = S // 2            # query rows per core
NKI = D // P           # 4 contraction chunks of 128
SPAD = S + (L - 1)     # 2052 zero-front-padded kf length
SQPAD = SQ + (L - 1)   # 1028 query halo length
BLK = 512              # pooling block (positions per block)
NCH = 5                # ctx chunks per pooling block: 4x128 + 4
NBK = S // BLK         # 4 kf pooling blocks
NBQ = SQ // BLK        # 2 q pooling blocks
NQC = SQ // BLK        # 2 SDPA query chunks of 512
NKC = S // P           # 16 SDPA key chunks of 128
RSQD = 1.0 / math.sqrt(D)
RSQK = 1.0 / math.sqrt(DK)
NCORES = 8

F32 = mybir.dt.float32
BF16 = mybir.dt.bfloat16

# Banded pooling geometry.  For 512-block t, context chunk m covers window
# rows (padded cols) [512t+128m, +K) = positions [512t+128m-4, +K), serving
# block-relative query cols [w0, w1).  Scores/exp/mask tiles pack the five
# windows as S1 = [m0|m1|m2] (392 cols) and S2 = [m3|m4] (136 cols).
#            m   s_off  w0   w1   K    tile
SEGS = [
    (0,   0,    0,  128, 128, 0),
    (1, 128,  124,  256, 128, 0),
    (2, 260,  252,  384, 128, 0),
    (3,   0,  380,  512, 128, 1),
    (4, 132,  508,  512,   4, 1),
]
S1W, S2W = 392, 136
# PSUM accumulation pieces per m: (lo, hi, start, stop) in block-rel cols.
# Adjacent windows overlap by 4 cols; boundary cols get two accumulating
# writes (start on first, stop on last).
PIECES = [
    [(0, 124, True, True), (124, 128, True, False)],
    [(124, 128, False, True), (128, 252, True, True), (252, 256, True, False)],
    [(252, 256, False, True), (256, 380, True, True), (380, 384, True, False)],
    [(380, 384, False, True), (384, 508, True, True), (508, 512, True, False)],
    [(508, 512, False, True)],
]

_PROG_CACHE = {}


def build_program(cfg=None):
    cfg = dict(cfg or {})
    MDT = BF16                       # matmul operand dtype everywhere

    nc = bacc.Bacc(
        "TRN2",
        target_bir_lowering=False,
        debug=False,
        enable_asserts=False,
        num_devices=NCORES,
    )

    keyT_d = nc.dram_tensor("keyT", [D, SPAD], MDT, kind="ExternalInput").ap()
    valT_d = nc.dram_tensor("valT", [D, S], MDT, kind="ExternalInput").ap()
    qT_d = nc.dram_tensor("qT", [D, SQPAD], MDT, kind="ExternalInput").ap()
    qrow_d = nc.dram_tensor("qrow", [SQPAD, D], MDT, kind="ExternalInput").ap()
    wfk_d = nc.dram_tensor("wfk", [D, D], MDT, kind="ExternalInput").ap()
    w0_d = nc.dram_tensor("w0", [D, D], MDT, kind="ExternalInput").ap()
    wout_d = nc.dram_tensor("wout", [D, D], MDT, kind="ExternalInput").ap()
    ones_d = nc.dram_tensor("ones_col", [P, 1], MDT, kind="ExternalInput").ap()
    vones_d = nc.dram_tensor("vones", [P, 2, 8 * H], MDT, kind="ExternalInput").ap()
    bfk_col_d = nc.dram_tensor("bfk_col", [D, 1], F32, kind="ExternalInput").ap()
    bfk_row_d = nc.dram_tensor("bfk_row", [1, D], F32, kind="ExternalInput").ap()
    b0_row_d = nc.dram_tensor("b0_row", [1, D], F32, kind="ExternalInput").ap()
    bout_col_d = nc.dram_tensor("bout_col", [D, 1], F32, kind="ExternalInput").ap()
    mask_d = nc.dram_tensor("mask_band", [P, S1W + S2W], BF16, kind="ExternalInput").ap()
    kfpad_d = nc.dram_tensor("kfpad", [D, L - 1], MDT, kind="ExternalInput").ap()
    outT_d = nc.dram_tensor("outT", [D, SQ], BF16, kind="ExternalOutput").ap()

    with tile.TileContext(nc) as tc:
        with (
            tc.tile_pool(name="A", bufs=4) as pA,      # keyT -> valT -> outT
            tc.tile_pool(name="Bp", bufs=4) as pB,     # kfT -> xt4
            tc.tile_pool(name="C", bufs=4) as pC,      # kfr -> qT/qrow -> v
            tc.tile_pool(name="W", bufs=2) as pW,      # wfk -> w0 -> wout
            tc.tile_pool(name="kTp", bufs=4) as pK,
            tc.tile_pool(name="qTp", bufs=4) as pQ,
            tc.tile_pool(name="small", bufs=1) as pS,
            tc.tile_pool(name="esc", bufs=12) as pE,   # pooling exp tiles
            tc.tile_pool(name="esb", bufs=6) as pEb,   # SDPA exp tiles
            tc.tile_pool(name="rec", bufs=14) as pR,   # recips/broadcasts/tmp
            tc.tile_pool(name="psS", bufs=2, space="PSUM") as psS,
            tc.tile_pool(name="psV", bufs=4, space="PSUM") as psV,
        ):

            # ---------------- pooling (banded attention, sw-pipelined) -------
            def emit_pool(xT_slice, xrow, out_tiles, nblocks, filler=None):
                """xT_slice(ki) -> [P, *PAD] transposed (padded) AP, bf16.
                xrow(n) -> (tile, idx) row-layout 128-row chunk n (bf16).
                out_tiles: 4 x [P, nblocks*BLK] bf16 pooled output (transposed).
                Banded: each m computes only its [K, w1-w0] window.
                Software-pipelined: scores(t+1) issue before den/PV(t)."""
                es_blocks = []

                def emit_scores(t):
                    Sp = [psS.tile([P, S1W], F32, tag="psS", name="S1"),
                          psS.tile([P, S2W], F32, tag="psS", name="S2")]
                    for (m, so, w0, w1, K, sel) in SEGS:
                        Ss = Sp[sel]
                        for ki in range(NKI):
                            xa = xT_slice(ki)
                            nc.tensor.matmul(
                                Ss[0:K, so : so + (w1 - w0)],
                                xa[:, BLK * t + P * m : BLK * t + P * m + K],
                                xa[:, L - 1 + BLK * t + w0 : L - 1 + BLK * t + w1],
                                start=(ki == 0),
                                stop=(ki == NKI - 1),
                            )
                    e1 = pE.tile([P, S1W], BF16, tag="esc", name="e1")
                    e2 = pE.tile([P, S2W], BF16, tag="esc", name="e2")
                    nc.scalar.activation(
                        e1[:], Sp[0][:], mybir.ActivationFunctionType.Exp, scale=RSQD)
                    nc.scalar.activation(
                        e2[:, 0:132], Sp[1][:, 0:132],
                        mybir.ActivationFunctionType.Exp, scale=RSQD)
                    nc.scalar.activation(
                        e2[0:4, 132:136], Sp[1][0:4, 132:136],
                        mybir.ActivationFunctionType.Exp, scale=RSQD)
                    nc.vector.tensor_mul(e1[:], e1[:], mask_sb[:, 0:S1W])
                    nc.vector.tensor_mul(
                        e2[:, 0:132], e2[:, 0:132], mask_sb[:, S1W : S1W + 132])
                    nc.vector.tensor_mul(
                        e2[0:4, 132:136], e2[0:4, 132:136],
                        mask_sb[0:4, S1W + 132 : S1W + 136])
                    return (e1, e2)

                def es_slice(es, m, lo, hi):
                    _, so, w0, _, K, sel = SEGS[m]
                    c0 = so + (lo - w0)
                    return es[sel][0:K, c0 : c0 + (hi - lo)]

                def emit_den(t, es):
                    dn = psV.tile([1, BLK], F32, tag="psV", name="den")
                    for m in range(NCH):
                        K = SEGS[m][4]
                        for (lo, hi, st, sp) in PIECES[m]:
                            nc.tensor.matmul(
                                dn[:, lo:hi],
                                ones_sb[0:K, :],
                                es_slice(es, m, lo, hi),
                                start=st,
                                stop=sp,
                            )
                    rc = pR.tile([1, BLK], F32, tag="rec")
                    nc.vector.reciprocal_approx_fast(rc[:], dn[:])
                    rb = pR.tile([P, BLK], F32, tag="rec")
                    nc.gpsimd.partition_broadcast(rb[:], rc[:])
                    return rb

                def emit_pv(t, es, rb):
                    for mo in range(NKI):
                        pv = psV.tile([P, BLK], F32, tag="psV", name=f"pv{mo}")
                        for m in range(NCH):
                            K = SEGS[m][4]
                            t_, idx = xrow(4 * t + m)
                            for (lo, hi, st, sp) in PIECES[m]:
                                nc.tensor.matmul(
                                    pv[:, lo:hi],
                                    t_[0:K, idx, P * mo : P * (mo + 1)],
                                    es_slice(es, m, lo, hi),
                                    start=st,
                                    stop=sp,
                                )
                        nc.vector.tensor_mul(
                            out_tiles[mo][:, BLK * t : BLK * (t + 1)], pv[:], rb[:]
                        )

                rbs = {}
                for t in range(nblocks):
                    if t > 0:
                        rbs[t - 1] = emit_den(t - 1, es_blocks[t - 1])
                    es_blocks.append(emit_scores(t))
                    if filler is not None:
                        filler(t)
                    if t > 0:
                        emit_pv(t - 1, es_blocks[t - 1], rbs[t - 1])
                rb_l = emit_den(nblocks - 1, es_blocks[nblocks - 1])
                emit_pv(nblocks - 1, es_blocks[nblocks - 1], rb_l)

            # ---------------- q loads first (the first pooling block waits
            # only on these), then the small constants ----------------
            qT_all = pC.tile([P, NKI, SQPAD], MDT, tag="C")
            qT_r = qT_d.rearrange("(t p) s -> p t s", p=P)
            nc.sync.dma_start(qT_all[:, :, 0:516], qT_r[:, :, 0:516])
            nc.sync.dma_start(qT_all[:, :, 516:SQPAD], qT_r[:, :, 516:SQPAD])
            qrowA = pC.tile([P, 9, BLK], MDT, tag="C")
            qrow_r = qrow_d[0:SQ, :].rearrange("(n p) d -> p n d", p=P)
            nc.sync.dma_start(qrowA[:, 0:4, :], qrow_r[:, 0:4, :])
            nc.sync.dma_start(qrowA[:, 4:8, :], qrow_r[:, 4:8, :])
            nc.sync.dma_start(qrowA[0:4, 8, :], qrow_d[SQ:SQPAD, :])

            # ---------------- constants / small loads ----------------
            mask_sb = pS.tile([P, S1W + S2W], BF16, tag="mask")
            nc.sync.dma_start(mask_sb[:], mask_d[:])
            bfk_col = pS.tile([P, NKI, 1], F32, tag="bfkc")
            nc.sync.dma_start(bfk_col[:], bfk_col_d.rearrange("(k p) o -> p k o", p=P))
            bout_col = pS.tile([P, NKI, 1], F32, tag="boutc")
            nc.sync.dma_start(bout_col[:], bout_col_d.rearrange("(k p) o -> p k o", p=P))
            ones_sb = pS.tile([P, 1], MDT, tag="ones")
            nc.sync.dma_start(ones_sb[:], ones_d[:])

            bfk_row = pS.tile([1, D], F32, tag="bfkrow")
            nc.sync.dma_start(bfk_row[:], bfk_row_d[:])
            bfk_bc = pS.tile([P, D], F32, tag="bfkbc")
            nc.gpsimd.partition_broadcast(bfk_bc[:], bfk_row[:])
            # variant with the 4 pad rows zeroed (for kf_row tile 0)
            bfk_bc0 = pS.tile([P, D], F32, tag="bfkbc0")
            nc.gpsimd.partition_broadcast(bfk_bc0[:], bfk_row[:])
            nc.vector.memset(bfk_bc0[0 : L - 1, :], 0.0)

            # ---------------- keyT + wfk loads (column-chunked) --------------
            keyT = [pA.tile([P, SPAD], MDT, tag="A", name=f"keyT{t}") for t in range(NKI)]
            bounds = [0, 516, 1028, 1540, SPAD]
            for cchunk in range(NKI):
                c0, c1 = bounds[cchunk], bounds[cchunk + 1]
                for t in range(NKI):
                    nc.sync.dma_start(keyT[t][:, c0:c1], keyT_d[P * t : P * (t + 1), c0:c1])
            wfk = pW.tile([P, NKI, D], MDT, tag="W")
            nc.sync.dma_start(wfk[:], wfk_d.rearrange("(k p) n -> p k n", p=P))

            qTp = [pQ.tile([P, SQ], MDT, tag="qTp", name=f"qTp{t}") for t in range(NKI)]
            emit_pool(lambda ki: qT_all[:, ki, :], lambda n: (qrowA, n), qTp, NBQ)

            # ---------------- kfT = (key @ W_fk + b_fk).T  [D, SPAD] ----------
            kfT = [pB.tile([P, SPAD], MDT, tag="B", name=f"kfT{t}") for t in range(NKI)]
            for mo in range(NKI):
                nc.sync.dma_start(kfT[mo][:, 0 : L - 1], kfpad_d[P * mo : P * (mo + 1), :])
            for ns in range(S // BLK):
                for mo in range(NKI):
                    ps = psS.tile([P, BLK], F32, tag="psS")
                    for ki in range(NKI):
                        nc.tensor.matmul(
                            ps[:],
                            wfk[:, ki, P * mo : P * (mo + 1)],
                            keyT[ki][:, L - 1 + BLK * ns : L - 1 + BLK * (ns + 1)],
                            start=(ki == 0),
                            stop=(ki == NKI - 1),
                        )
                    nc.scalar.add(
                        kfT[mo][:, L - 1 + BLK * ns : L - 1 + BLK * (ns + 1)],
                        ps[:],
                        bfk_col[:, mo, :],
                    )

            # ---------------- kf_row  [SPAD rows, D]  (17 x 128-row tiles) ----
            kfrA = pC.tile([P, 9, BLK], MDT, tag="C")
            kfrB = pC.tile([P, 8, BLK], MDT, tag="C")

            def kfr(n):
                return (kfrA, n) if n < 9 else (kfrB, n - 9)

            NROW = SPAD // P + 1  # 17
            for n in range(NROW):
                M = P if n < NROW - 1 else SPAD - P * (NROW - 1)  # 128 or 4
                ps = psS.tile([P, BLK], F32, tag="psS")
                for ki in range(NKI):
                    nc.tensor.matmul(
                        ps[0:M, :],
                        keyT[ki][:, P * n : P * n + M],
                        wfk[:, ki, :],
                        start=(ki == 0),
                        stop=(ki == NKI - 1),
                    )
                t_, idx = kfr(n)
                bias = bfk_bc0 if n == 0 else bfk_bc
                nc.vector.tensor_add(t_[0:M, idx, :], ps[0:M, :], bias[0:M, :])

            # ---------------- v = value @ W0 + b0 prep (emitted inside kfpool)
            w0 = pW.tile([P, NKI, D], MDT, tag="W")
            nc.sync.dma_start(w0[:], w0_d.rearrange("(k p) n -> p k n", p=P))
            valT = [pA.tile([P, S], MDT, tag="A", name=f"valT{t}") for t in range(NKI)]
            for t in range(NKI):
                nc.sync.dma_start(valT[t][:], valT_d[P * t : P * (t + 1), :])
            b0_row = pS.tile([1, D], F32, tag="b0row")
            nc.sync.dma_start(b0_row[:], b0_row_d[:])
            b0_bc = pS.tile([P, D], F32, tag="b0bc")
            nc.gpsimd.partition_broadcast(b0_bc[:], b0_row[:])

            vA = pC.tile([P, 8, H, DK + 1], MDT, tag="C")
            vB = pC.tile([P, 8, H, DK + 1], MDT, tag="C")
            nc.sync.dma_start(vA[:, :, :, DK], vones_d[:, 0, :].rearrange("p (n h) -> p n h", n=8))
            nc.sync.dma_start(vB[:, :, :, DK], vones_d[:, 1, :].rearrange("p (n h) -> p n h", n=8))

            def emit_v_chunk(n):
                ps = psS.tile([P, BLK], F32, tag="psS")
                for ki in range(NKI):
                    nc.tensor.matmul(
                        ps[:],
                        valT[ki][:, P * n : P * (n + 1)],
                        w0[:, ki, :],
                        start=(ki == 0),
                        stop=(ki == NKI - 1),
                    )
                vt = vA if n < 8 else vB
                nc.vector.tensor_add(
                    vt[:, n % 8, :, 0:DK],
                    ps[:].rearrange("p (h z) -> p h z", h=H),
                    b0_bc[:].rearrange("p (h z) -> p h z", h=H),
                )

            kTp = [pK.tile([P, S], MDT, tag="kTp", name=f"kTp{t}") for t in range(NKI)]

            def v_filler(t):
                for n in range(4 * t, 4 * t + 4):
                    emit_v_chunk(n)

            emit_pool(lambda ki: kfT[ki][:], kfr, kTp, NBK, filler=v_filler)

            # ---------------- SDPA + output projection ----------------
            wout = pW.tile([P, NKI, D], MDT, tag="W")
            nc.sync.dma_start(wout[:], wout_d.rearrange("(k p) n -> p k n", p=P))
            xt4 = [pB.tile([P, SQ], MDT, tag="B", name=f"xt4_{t}") for t in range(NKI)]
            outT = [pA.tile([P, SQ], BF16, tag="A", name=f"outT{t}") for t in range(NKI)]

            def emit_outproj(qc):
                for mo in range(NKI):
                    po = psS.tile([P, BLK], F32, tag="psS")
                    for ki in range(NKI):
                        nc.tensor.matmul(
                            po[:],
                            wout[:, ki, P * mo : P * (mo + 1)],
                            xt4[ki][:, BLK * qc : BLK * (qc + 1)],
                            start=(ki == 0),
                            stop=(ki == NKI - 1),
                        )
                    nc.vector.tensor_scalar_add(
                        outT[mo][:, BLK * qc : BLK * (qc + 1)], po[:], bout_col[:, mo, :]
                    )
                    nc.sync.dma_start(
                        outT_d[P * mo : P * (mo + 1), BLK * qc : BLK * (qc + 1)],
                        outT[mo][:, BLK * qc : BLK * (qc + 1)],
                    )

            if cfg.get("v2_sdpa"):
                # v2 SDPA path (sequential heads, DVE reciprocal) for bisection
                for h in range(H):
                    th, off = h // 2, DK * (h % 2)
                    pxs = [psV.tile([DK + 1, BLK], F32, tag="psV", name=f"px{qc}")
                           for qc in range(NQC)]
                    for kc in range(NKC):
                        ps = psS.tile([P, NQC * BLK], F32, tag="psS")
                        for qc in range(NQC):
                            nc.tensor.matmul(
                                ps[:, BLK * qc : BLK * (qc + 1)],
                                kTp[th][off : off + DK, P * kc : P * (kc + 1)],
                                qTp[th][off : off + DK, BLK * qc : BLK * (qc + 1)],
                                start=True,
                                stop=True,
                            )
                        e = pEb.tile([P, NQC * BLK], BF16, tag="esb")
                        nc.scalar.activation(
                            e[:], ps[:], mybir.ActivationFunctionType.Exp, scale=RSQK
                        )
                        vt = vA if kc < 8 else vB
                        for qc in range(NQC):
                            nc.tensor.matmul(
                                pxs[qc][:],
                                vt[:, kc % 8, h, :],
                                e[:, BLK * qc : BLK * (qc + 1)],
                                start=(kc == 0),
                                stop=(kc == NKC - 1),
                            )
                    def norm_qc2(qc):
                        px = pxs[qc]
                        rc = pR.tile([DK + 1, BLK], F32, tag="rec")
                        nc.vector.reciprocal(rc[DK : DK + 1, :], px[DK : DK + 1, :])
                        rc0 = pR.tile([1, BLK], F32, tag="rec")
                        nc.sync.dma_start(rc0[:], rc[DK : DK + 1, :])
                        rb = pR.tile([DK, BLK], F32, tag="rec")
                        nc.gpsimd.partition_broadcast(rb[:], rc0[:])
                        if h % 2 == 0:
                            nc.vector.tensor_mul(
                                xt4[th][0:DK, BLK * qc : BLK * (qc + 1)], px[0:DK, :], rb[:]
                            )
                        else:
                            tmp = pR.tile([DK, BLK], MDT, tag="rec")
                            nc.vector.tensor_mul(tmp[:], px[0:DK, :], rb[:])
                            nc.sync.dma_start(
                                xt4[th][DK:P, BLK * qc : BLK * (qc + 1)], tmp[:]
                            )
                    norm_qc2(0)
                    if h == H - 1:
                        emit_outproj(0)
                    norm_qc2(1)
                emit_outproj(1)

            # Head pairs (2p, 2p+1): lhsT/rhs slices at base partitions 0/64
            # land in disjoint PE row-groups -> the two 64-contraction score
            # matmuls run concurrently (tile_position auto-derived).
            for p in range(4 if not cfg.get("v2_sdpa") else 0):
                pxs = {}
                for hh in range(2):
                    for qc in range(NQC):
                        pxs[(hh, qc)] = psV.tile(
                            [DK + 1, BLK], F32, tag="psV", name=f"px{hh}{qc}")
                for kc in range(NKC):
                    vt = vA if kc < 8 else vB
                    for qc in range(NQC):
                        ps = psS.tile([P, 2 * BLK], F32, tag="psS")
                        nc.tensor.matmul(
                            ps[:, 0:BLK],
                            kTp[p][0:DK, P * kc : P * (kc + 1)],
                            qTp[p][0:DK, BLK * qc : BLK * (qc + 1)],
                            start=True,
                            stop=True,
                        )
                        nc.tensor.matmul(
                            ps[:, BLK : 2 * BLK],
                            kTp[p][DK:P, P * kc : P * (kc + 1)],
                            qTp[p][DK:P, BLK * qc : BLK * (qc + 1)],
                            start=True,
                            stop=True,
                        )
                        e = pEb.tile([P, 2 * BLK], BF16, tag="esb")
                        nc.scalar.activation(
                            e[:], ps[:], mybir.ActivationFunctionType.Exp, scale=RSQK
                        )
                        nc.tensor.matmul(
                            pxs[(0, qc)][:],
                            vt[:, kc % 8, 2 * p, :],
                            e[:, 0:BLK],
                            start=(kc == 0),
                            stop=(kc == NKC - 1),
                        )
                        nc.tensor.matmul(
                            pxs[(1, qc)][:],
                            vt[:, kc % 8, 2 * p + 1, :],
                            e[:, BLK : 2 * BLK],
                            start=(kc == 0),
                            stop=(kc == NKC - 1),
                        )

                # Evacuate the PV accumulators to SBUF immediately: frees the
                # 4 PSUM slots for the next pair's accumulation while the
                # (latency-bound) norm chains run from SBUF.
                xsb = {}
                for qc in range(NQC):
                    for hh in range(2):
                        t_ = pR.tile([DK + 1, BLK], F32, tag="rec")
                        nc.vector.tensor_copy(t_[:], pxs[(hh, qc)][:])
                        xsb[(hh, qc)] = t_

                def norm_qc(hh, qc):
                    px = xsb[(hh, qc)]
                    # custom-ucode DVE/gpsimd ops address tile partition 0,
                    # not the AP base -- DMA the den row down to a base-0
                    # tile, then reciprocal + broadcast at base 0.
                    dn0 = pR.tile([1, BLK], F32, tag="rec")
                    nc.sync.dma_start(dn0[:], px[DK : DK + 1, :])
                    rc0 = pR.tile([1, BLK], F32, tag="rec")
                    nc.vector.reciprocal_approx_fast(rc0[:], dn0[:])
                    rb = pR.tile([DK, BLK], F32, tag="rec")
                    nc.gpsimd.partition_broadcast(rb[:], rc0[:])
                    if hh == 0:
                        nc.vector.tensor_mul(
                            xt4[p][0:DK, BLK * qc : BLK * (qc + 1)], px[0:DK, :], rb[:]
                        )
                    else:
                        tmp = pR.tile([DK, BLK], MDT, tag="rec")
                        nc.vector.tensor_mul(tmp[:], px[0:DK, :], rb[:])
                        nc.sync.dma_start(
                            xt4[p][DK:P, BLK * qc : BLK * (qc + 1)], tmp[:]
                        )

                norm_qc(0, 0)
                norm_qc(1, 0)
                if p == 3:
                    emit_outproj(0)
                norm_qc(0, 1)
                norm_qc(1, 1)
            if not cfg.get("v2_sdpa"):
                emit_outproj(1)

    nc.compile()
    return nc


def make_band_mask():
    """[128, 528] bf16: per-segment band masks, S1 | S2 packed layout."""
    M = np.zeros((P, S1W + S2W), np.float32)
    off = {0: 0, 1: S1W}
    for (m, so, w0, w1, K, sel) in SEGS:
        for r in range(K):
            j = 128 * m - 4 + r
            for c in range(w1 - w0):
                i = w0 + c
                if 0 <= i - j <= L - 1:
                    M[r, off[sel] + so + c] = 1.0
    return M


def make_core_inputs(query, key, value, W_fk, b_fk, W0, b0, Wout, bout, cfg=None):
    """Build the 8 per-core input dicts from full inputs (host-side shard)."""
    bf = ml_dtypes.bfloat16
    shared = {
        "wfk": np.ascontiguousarray(W_fk).astype(bf),
        "w0": np.ascontiguousarray(W0).astype(bf),
        "wout": np.ascontiguousarray(Wout).astype(bf),
        "ones_col": np.ones((P, 1), bf),
        "vones": np.ones((P, 2, 8 * H), bf),
        "bfk_col": np.ascontiguousarray(b_fk.reshape(D, 1), np.float32),
        "bfk_row": np.ascontiguousarray(b_fk.reshape(1, D), np.float32),
        "b0_row": np.ascontiguousarray(b0.reshape(1, D), np.float32),
        "bout_col": np.ascontiguousarray(bout.reshape(D, 1), np.float32),
        "mask_band": make_band_mask().astype(bf),
        "kfpad": np.zeros((D, L - 1), bf),
    }
    in_maps = []
    for c in range(NCORES):
        b, half = divmod(c, 2)
        q0 = half * SQ
        q_halo = np.zeros((SQPAD, D), np.float32)
        lo = max(0, q0 - (L - 1))
        q_halo[(L - 1) - (q0 - lo):] = query[b, lo : q0 + SQ]
        keyT_pad = np.zeros((D, SPAD), np.float32)
        keyT_pad[:, L - 1 :] = key[b].T
        m = dict(shared)
        m["keyT"] = keyT_pad.astype(bf)
        m["valT"] = np.ascontiguousarray(value[b].T).astype(bf)
        m["qT"] = np.ascontiguousarray(q_halo.T).astype(bf)
        m["qrow"] = q_halo.astype(bf)
        in_maps.append(m)
    return in_maps


def _cfg_from_env():
    cfg = {}
    if os.environ.get("K_V2_SDPA"):
        cfg["v2_sdpa"] = True
    return cfg


def get_program(cfg=None):
    cfg = dict(cfg or {})
    key_t = tuple(sorted(cfg.items()))
    if key_t not in _PROG_CACHE:
        _PROG_CACHE[key_t] = build_program(cfg)
    return _PROG_CACHE[key_t]


def kernel(query, key, value, mask=None, W_fk=None, b_fk=None, W0=None, b0=None,
           Wout=None, bout=None, **extra):
    del mask, extra  # mask is dead in the reference (forward passes mask=None)
    cfg = _cfg_from_env()
    nc = get_program(cfg)

    query = np.asarray(query, np.float32)
    key = np.asarray(key, np.float32)
    value = np.asarray(value, np.float32)
    in_maps = make_core_inputs(
        query, key, value,
        np.asarray(W_fk, np.float32), np.asarray(b_fk, np.float32),
        np.asarray(W0, np.float32), np.asarray(b0, np.float32),
        np.asarray(Wout, np.float32), np.asarray(bout, np.float32),
        cfg,
    )
    res = bass_utils.run_bass_kernel_spmd(nc, in_maps, core_ids=list(range(NCORES)))
    out = np.empty((B, S, D), np.float32)
    for c in range(NCORES):
        b, half = divmod(c, 2)
        out[b, half * SQ : (half + 1) * SQ, :] = res.results[c]["outT"].astype(np.float32).T
    return out
